# revision 52
# baseline (speedup 1.0000x reference)
"""EdgeNetworkLayer Trainium2 kernel: 8-core SPMD, edges sharded BY TARGET.

Core c owns nodes [c*1024, (c+1)*1024) and every edge pointing into them, so
the per-shard segment_sum is complete locally and NO collective is needed;
each core runs the GRU on its own node shard and returns it.

messages[e,i] = sum_{k,j} z[e,k] * h_w[e,j] * W2[k, i*128+j]
with z = relu(ef @ W1 + b1); the bilinear form is one PE matmul chain with
contraction dim (k,j) = 64*128 = 8192:
  msgT[i, e] = sum_t W2p_t[p, i].T @ PT_t[p, e]
where tile t = (g, b), partition p = (a, c), k = 4g+a, j = 32b+c,
PT_t[p, e] = z[e, 4g+a] * h_w[e, 32b+c] (built on DVE in fp16, the pacing
engine). The h side (h_w gather, transpose, x4 block replication) is pure
host-known data movement and is shipped as inputs (hwT, H32) like the
one-hot scatter matrix; z is computed on device and DMA-replicated x32 per
chunk from a DRAM bounce (SBUF sources can't partition-broadcast), one op
per (chunk, a) across four rings with a 2-chunk lead.

Edges are processed in 512-column chunks; scatter + GRU + msg transposes are
interleaved one-op-per-g into the main loop's PE gaps so the PE stays
continuously busy (full p-state) while DVE paces.

Scatter: per node tile, PSUM-chained matmuls with stationary = msg tile
(fp16, PE transpose of msgT) and moving = band-limited one-hot S (exact 0/1
fp16), producing mT [i, node] directly. Z and GRU matmuls in float32r.
"""
import numpy as np

N, H, E, ED, MLP_HID = 8192, 128, 16384, 16, 64
NCORES = 8
P = 128
NS = N // NCORES          # 1024 nodes per core
NT = NS // P              # 8 node tiles per core
KG = 16                   # k-groups of 4
GRU_COLS = 256            # GRU column-group width (>=256 for f32r fast path)


def _host_prep(h, edge_index, edge_features, W1, b1, W2, b2, W_ih, W_hh, b_ih, b_hh):
    f32, f16 = np.float32, np.float16
    h = np.ascontiguousarray(h, f32)
    src_all = np.asarray(edge_index[0], np.int64)
    tgt_all = np.asarray(edge_index[1], np.int64)
    ef_all = np.asarray(edge_features, f32)

    order = np.argsort(tgt_all, kind="stable")
    s_s, t_s, ef_s = src_all[order], tgt_all[order], ef_all[order]
    shard_of = t_s // NS
    shards = []
    for c in range(NCORES):
        m = shard_of == c
        shards.append((s_s[m], t_s[m] - c * NS, ef_s[m]))
    CAP = ((max(len(s[0]) for s in shards) + P - 1) // P) * P
    ET = CAP // P

    # tile band plan, uniform across cores
    base = np.zeros(ET, np.int64)
    endv = np.zeros(ET, np.int64)
    any_real = np.zeros(ET, bool)
    for ti in range(ET):
        lo, hi = NS, 0
        for c in range(NCORES):
            seg = shards[c][1][ti * P:(ti + 1) * P]
            if len(seg):
                any_real[ti] = True
                lo = min(lo, int(seg.min()))
                hi = max(hi, int(seg.max()) + 1)
        if any_real[ti]:
            base[ti] = (lo // P) * P
            endv[ti] = hi
    W_band = P
    for ti in range(ET):
        if any_real[ti]:
            W_band = max(W_band, int(-((base[ti] - endv[ti]) // P)) * P)

    # contrib[ng] = edge tiles feeding node tile ng (union over cores)
    contrib = [[] for _ in range(NT)]
    for ti in range(ET):
        if not any_real[ti]:
            continue
        ngs = set()
        for c in range(NCORES):
            seg = shards[c][1][ti * P:(ti + 1) * P]
            if len(seg):
                ngs |= set(int(x) for x in np.unique(seg // P))
        for ng in sorted(ngs):
            contrib[ng].append(ti)

    hf16 = h.astype(f16)
    # (a=8, c=16) factorization: k = 8*gg + a, j = 16*bb + c, p = 16a + c
    W2r = np.asarray(W2, f32).reshape(MLP_HID, H, H)            # [k, i, j]
    W2g = W2r.reshape(8, 8, H, 8, 16)                           # [gg, a, i, bb, c]
    W2p = W2g.transpose(0, 3, 1, 4, 2).reshape(64, P, H)        # [(gg,bb), (a,c), i]
    W2P_host = np.ascontiguousarray(W2p.transpose(1, 0, 2).astype(f16))  # [p, 64, i]
    # [c, bb, i]: b2 tile pre-split for 16-row matmuls against H16 bb-slices
    W2b2_host = np.ascontiguousarray(
        np.asarray(b2, f32).reshape(H, 8, 16).transpose(2, 1, 0).astype(f16))
    W1p = np.concatenate([np.asarray(W1, f32), np.asarray(b1, f32)[None, :]], 0)

    W_ihT = np.ascontiguousarray(np.asarray(W_ih, f32).T)       # [128, 384]
    W_hhT = np.ascontiguousarray(np.asarray(W_hh, f32).T)
    b_ih = np.asarray(b_ih, f32)
    b_hh = np.asarray(b_hh, f32)
    b_r = (b_ih[:H] + b_hh[:H]).reshape(H, 1).astype(f32)
    b_z = (b_ih[H:2 * H] + b_hh[H:2 * H]).reshape(H, 1).astype(f32)
    b_in = b_ih[2 * H:].reshape(H, 1).astype(f32)
    b_hn = b_hh[2 * H:].reshape(H, 1).astype(f32)

    in_maps = []
    for c in range(NCORES):
        s, toff, ef = shards[c]
        n = len(s)
        s_pad = np.zeros(CAP, np.int64)
        s_pad[:n] = s
        ef_pad = np.zeros((CAP, ED), f32)
        ef_pad[:n] = ef
        efT = np.concatenate([ef_pad.T, np.ones((1, CAP), f32)], 0)   # [17, CAP]
        hwT = np.ascontiguousarray(hf16[s_pad].T)                     # [128, CAP]
        H32 = np.empty((P, 8, CAP), f16)
        for a in range(8):
            for b in range(8):
                H32[16 * a:16 * a + 16, b, :] = hwT[16 * b:16 * b + 16, :]
        Sband = np.zeros((P, ET, W_band), f16)
        idx = np.arange(n)
        Sband[idx % P, idx // P, toff - base[idx // P]] = 1.0
        hTs = np.ascontiguousarray(h[c * NS:(c + 1) * NS].T)          # [128, 1024]
        in_maps.append(dict(
            efT=efT, H32=H32, Sband=Sband, W2P=W2P_host,
            W2b2=W2b2_host, W1p=W1p, WihT=W_ihT, WhhT=W_hhT, b_r=b_r,
            b_z=b_z, b_in=b_in, b_hn=b_hn, hTs=hTs))
    return (in_maps, CAP, W_band, tuple(int(b) for b in base),
            tuple(tuple(cc) for cc in contrib))


def _build_program(CAP, W_band, base_arr, contrib):
    import concourse.bass as bass
    import concourse.bacc as bacc
    import concourse.tile as tile
    import concourse.mybir as mybir
    from concourse.masks import make_identity

    dt = mybir.dt.float32
    f16 = mybir.dt.float16
    f32r = mybir.dt.float32r
    AF = mybir.ActivationFunctionType
    OP = mybir.AluOpType

    ET = CAP // P
    chunks = []
    t0 = 0
    while t0 < ET:
        nt_ = min(4, ET - t0)
        chunks.append((t0, nt_))
        t0 += nt_
    NCH = len(chunks)
    last_tile_of_chunk = [t0_ + nt_ - 1 for (t0_, nt_) in chunks]

    ready = [[] for _ in range(NCH)]
    empty_ng = []
    for ng in range(NT):
        if not contrib[ng]:
            empty_ng.append(ng)
            continue
        need = max(contrib[ng])
        for ci in range(NCH):
            if need <= last_tile_of_chunk[ci]:
                ready[ci].append(ng)
                break
    ngrp = GRU_COLS // P
    ng_done_at = {ng: 0 for ng in empty_ng}
    for ci in range(NCH):
        for ng in ready[ci]:
            ng_done_at[ng] = ci
    gru_ready = [[] for _ in range(NCH)]
    for gg in range(NS // GRU_COLS):
        ci = max(ng_done_at[gg * ngrp + i] for i in range(ngrp))
        gru_ready[ci].append(gg)

    nc = bacc.Bacc("TRN2", target_bir_lowering=False, debug=False,
                   num_devices=NCORES)

    efT_d = nc.dram_tensor("efT", [ED + 1, CAP], f32r, kind="ExternalInput")
    H32_d = nc.dram_tensor("H32", [P, 8, CAP], f16, kind="ExternalInput")
    S_d = nc.dram_tensor("Sband", [P, ET, W_band], f16, kind="ExternalInput")
    W2P_d = nc.dram_tensor("W2P", [P, 64, H], f16, kind="ExternalInput")
    W2b2_d = nc.dram_tensor("W2b2", [16, 8, H], f16, kind="ExternalInput")
    W1p_d = nc.dram_tensor("W1p", [ED + 1, MLP_HID], f32r, kind="ExternalInput")
    WihT_d = nc.dram_tensor("WihT", [H, 3 * H], f32r, kind="ExternalInput")
    WhhT_d = nc.dram_tensor("WhhT", [H, 3 * H], f32r, kind="ExternalInput")
    br_d = nc.dram_tensor("b_r", [H, 1], dt, kind="ExternalInput")
    bz_d = nc.dram_tensor("b_z", [H, 1], dt, kind="ExternalInput")
    bin_d = nc.dram_tensor("b_in", [H, 1], dt, kind="ExternalInput")
    bhn_d = nc.dram_tensor("b_hn", [H, 1], dt, kind="ExternalInput")
    hTs_d = nc.dram_tensor("hTs", [H, NS], f32r, kind="ExternalInput")
    out_d = nc.dram_tensor("out_hT", [H, NS], dt, kind="ExternalOutput")

    with tile.TileContext(nc) as tc:
        with (
            tc.tile_pool(name="const", bufs=1) as cp,
            tc.tile_pool(name="dram", bufs=1, space="DRAM") as dram,
            tc.tile_pool(name="work", bufs=1) as wp,
        ):
            # ---------- startup
            ident16 = cp.tile([P, P], f16)
            make_identity(nc, ident16[:])
            efT = cp.tile([ED + 1, CAP], f32r)
            W1p = cp.tile([ED + 1, MLP_HID], f32r)
            nc.sync.dma_start(W1p[:], W1p_d[:])
            for (t0_, nt_) in chunks:
                c0, cw = t0_ * P, nt_ * P
                nc.sync.dma_start(efT[:, c0:c0 + cw], efT_d[:, c0:c0 + cw])

            H32 = cp.tile([P, 8, CAP], f16)
            c0w = chunks[0][1] * P
            nc.gpsimd.dma_start(H32[:, :, :c0w], H32_d[:, :, :c0w])

            W2P = cp.tile([P, 64, H], f16)
            w2b2 = cp.tile([16, 8, H], f16)
            WihT = cp.tile([H, 3 * H], f32r)
            WhhT = cp.tile([H, 3 * H], f32r)
            b_r = cp.tile([H, 1], dt)
            b_z = cp.tile([H, 1], dt)
            b_in = cp.tile([H, 1], dt)
            b_hn = cp.tile([H, 1], dt)
            hTs = cp.tile([H, NS], f32r)
            Sband = cp.tile([P, ET, W_band], f16)

            zT = wp.tile([MLP_HID, CAP], f16)
            zT_dram = dram.tile([MLP_HID, CAP], f16)

            # Z: zT = relu(W1p.T @ efT), chunked, bounced to DRAM
            with tc.tile_pool(name="psz", bufs=2, space="PSUM") as psz:
                zc0 = 0
                while zc0 < CAP:
                    cw = min(512, CAP - zc0)
                    zps = psz.tile([MLP_HID, 512], dt, tag="zps")
                    nc.tensor.matmul(zps[:, :cw], W1p[:],
                                     efT[:, zc0:zc0 + cw],
                                     start=True, stop=True)
                    nc.scalar.activation(zT[:, zc0:zc0 + cw], zps[:, :cw],
                                         AF.Relu)
                    nc.scalar.dma_start(zT_dram[:, zc0:zc0 + cw],
                                        zT[:, zc0:zc0 + cw])
                    zc0 += cw

            def emit_z32(ci, slot, engines, fine=False):
                (t0_, nt_) = chunks[ci]
                c0, cw = t0_ * P, nt_ * P
                zv = zT_dram[:, c0:c0 + cw].rearrange("(g a) e -> a g e", a=8)
                if fine:
                    k = 0
                    for gp in range(4):
                        for a in range(8):
                            eng = engines[k % len(engines)]
                            k += 1
                            eng.dma_start(
                                slot[16 * a:16 * a + 16,
                                     2 * gp:2 * gp + 2, :cw],
                                zv[a:a + 1, 2 * gp:2 * gp + 2]
                                .broadcast_to((16, 2, cw)))
                else:
                    for a in range(8):
                        eng = engines[a % len(engines)]
                        eng.dma_start(
                            slot[16 * a:16 * a + 16, :, :cw],
                            zv[a:a + 1].broadcast_to((16, 8, cw)))

            # ---------- main pipeline
            msgT16 = wp.tile([P, CAP], f16)
            msg = wp.tile([P, ET, P], f16)
            mT = wp.tile([H, NS], f32r)
            out_sb = wp.tile([H, NS], dt)
            for ng in empty_ng:
                nc.gpsimd.memset(mT[:, ng * P:(ng + 1) * P], 0.0)

            with (
                tc.tile_pool(name="psacc", bufs=2, space="PSUM") as psacc,
                tc.tile_pool(name="pstr", bufs=2, space="PSUM") as pstr,
                tc.tile_pool(name="psm", bufs=2, space="PSUM") as psm,
                tc.tile_pool(name="psg", bufs=1, space="PSUM") as psg,
                tc.tile_pool(name="zpool", bufs=1) as zpool,
                tc.tile_pool(name="ptpool", bufs=3) as ptpool,
            ):
                def scat_ng(ng):
                    pm = psm.tile([P, P], dt, tag="pm")
                    for idx, ti in enumerate(contrib[ng]):
                        off = ng * P - int(base_arr[ti])
                        nc.tensor.matmul(
                            pm[:], msg[:, ti, :], Sband[:, ti, off:off + P],
                            start=(idx == 0),
                            stop=(idx == len(contrib[ng]) - 1))
                    nc.scalar.copy(mT[:, ng * P:(ng + 1) * P], pm[:])

                def msg_transpose(t):
                    tp = pstr.tile([P, P], f16, tag="tp")
                    nc.tensor.transpose(
                        tp[:], msgT16[:, t * P:(t + 1) * P], ident16[:])
                    nc.scalar.copy(msg[:, t, :], tp[:])

                def gru_group(gg):
                    ew = nc.vector if gg >= (NS // GRU_COLS) - 2 else nc.gpsimd
                    c0 = gg * GRU_COLS
                    csl = slice(c0, c0 + GRU_COLS)
                    rz_ps = psg.tile([H, 2, GRU_COLS], dt, tag="rzp")
                    nn_ps = psg.tile([H, 2, GRU_COLS], dt, tag="nnp")
                    gin_ps = nn_ps[:, 0, :]
                    ghn_ps = nn_ps[:, 1, :]
                    for q in range(2):
                        nc.tensor.matmul(rz_ps[:, q, :],
                                         WihT[:, q * H:(q + 1) * H],
                                         mT[:, csl], start=True, stop=False)
                        nc.tensor.matmul(rz_ps[:, q, :],
                                         WhhT[:, q * H:(q + 1) * H],
                                         hTs[:, csl], start=False, stop=True)
                    nc.tensor.matmul(gin_ps, WihT[:, 2 * H:3 * H],
                                     mT[:, csl], start=True, stop=True)
                    nc.tensor.matmul(ghn_ps, WhhT[:, 2 * H:3 * H],
                                     hTs[:, csl], start=True, stop=True)
                    rz = wp.tile([H, 2, GRU_COLS], dt, tag=f"rz{gg % 2}")
                    nc.scalar.activation(rz[:, 0, :], rz_ps[:, 0, :],
                                         AF.Sigmoid, bias=b_r[:])
                    nc.scalar.activation(rz[:, 1, :], rz_ps[:, 1, :],
                                         AF.Sigmoid, bias=b_z[:])
                    ghn = wp.tile([H, GRU_COLS], dt, tag=f"ghn{gg % 2}")
                    nc.scalar.activation(ghn[:], ghn_ps, AF.Identity,
                                         bias=b_hn[:])
                    ew.tensor_mul(ghn[:], rz[:, 0, :], ghn[:])
                    nc.vector.tensor_add(ghn[:], ghn[:], gin_ps)
                    ng_ = wp.tile([H, GRU_COLS], dt, tag=f"ng{gg % 2}")
                    nc.scalar.activation(ng_[:], ghn[:], AF.Tanh, bias=b_in[:])
                    dif = wp.tile([H, GRU_COLS], dt, tag=f"dif{gg % 2}")
                    ew.tensor_sub(dif[:], hTs[:, csl].bitcast(dt), ng_[:])
                    ew.tensor_mul(dif[:], rz[:, 1, :], dif[:])
                    ew.tensor_add(out_sb[:, csl], ng_[:], dif[:])
                    nc.sync.dma_start(out_d[:, csl], out_sb[:, csl])

                # startup tail: chunk-0/1 z32 first on each ring (FIFO),
                # then bulk loads behind them.
                zslot0 = zpool.tile([P, 8, 512], f16, tag="zs0")
                zslot1 = zpool.tile([P, 8, 512], f16, tag="zs1")
                zslot2 = zpool.tile([P, 8, 512], f16, tag="zs2")
                zslots = [zslot0, zslot1, zslot2]
                emit_z32(0, zslots[0], [nc.sync, nc.scalar, nc.gpsimd],
                         fine=True)

                # Sequencing: engine queues issue in order, so tiny token
                # DMAs (reading a just-landed z32 region, f16-matched) hold
                # bulk loads behind the chunk-0/1 critical z32 stream.
                def tok_dma(eng, dst, slot_i, g_idx):
                    eng.dma_start(dst, zslots[slot_i][96:97, g_idx, 0:1])

                nc.sync.dma_start(W2P[:, 0:16, :], W2P_d[:, 0:16, :])
                if NCH > 1:
                    emit_z32(1, zslots[1], [nc.sync, nc.scalar, nc.gpsimd],
                             fine=True)
                tok_dma(nc.sync, W2P[0:1, 16, 0:1], 0, 3)
                nc.sync.dma_start(W2P[:, 16:32, :], W2P_d[:, 16:32, :])
                tok_dma(nc.sync, W2P[0:1, 32, 0:1], 0, 5)
                nc.sync.dma_start(W2P[:, 32:48, :], W2P_d[:, 32:48, :])
                tok_dma(nc.sync, W2P[0:1, 48, 0:1], 0, 7)
                nc.sync.dma_start(W2P[:, 48:64, :], W2P_d[:, 48:64, :])
                t00 = chunks[0][0]
                tok_dma(nc.sync, Sband[0:1, t00, 0:1], 1 if NCH > 1 else 0, 1)
                nc.sync.dma_start(
                    Sband[:, t00:t00 + chunks[0][1], :],
                    S_d[:, t00:t00 + chunks[0][1], :])
                nc.sync.dma_start(w2b2[:], W2b2_d[:])
                nc.sync.dma_start(WihT[:], WihT_d[:])
                nc.sync.dma_start(WhhT[:], WhhT_d[:])
                nc.sync.dma_start(hTs[:], hTs_d[:])
                nc.sync.dma_start(b_r[:], br_d[:])
                nc.sync.dma_start(b_z[:], bz_d[:])
                nc.sync.dma_start(b_in[:], bin_d[:])
                nc.sync.dma_start(b_hn[:], bhn_d[:])
                # H32 rest, gated + progressive, on the gpsimd queue
                tok_dma(nc.gpsimd, H32[0:1, 0, c0w:c0w + 1], 0, 5)
                if CAP > 1024 and NCH > 2:
                    nc.gpsimd.dma_start(H32[:, :, c0w:1024],
                                        H32_d[:, :, c0w:1024])
                    prev = 1024
                    for ci_ in range(2, NCH):
                        cend = min((chunks[ci_][0] + chunks[ci_][1]) * P, CAP)
                        tok_dma(nc.gpsimd, H32[0:1, 0, prev:prev + 1],
                                1, 2 * ci_ - 3 if 2 * ci_ - 3 < 8 else 7)
                        nc.gpsimd.dma_start(H32[:, :, prev:cend],
                                            H32_d[:, :, prev:cend])
                        prev = cend
                else:
                    nc.gpsimd.dma_start(H32[:, :, c0w:], H32_d[:, :, c0w:])

                deferred = []
                for ci, (t0_, nt_) in enumerate(chunks):
                    c0, cw = t0_ * P, nt_ * P
                    if ci + 2 < NCH:
                        emit_z32(ci + 2, zslots[(ci + 2) % 3],
                                 [nc.sync, nc.scalar])
                    if ci + 1 < NCH:
                        t1_, n1_ = chunks[ci + 1]
                        nc.sync.dma_start(Sband[:, t1_:t1_ + n1_, :],
                                          S_d[:, t1_:t1_ + n1_, :])
                    acc = psacc.tile([P, 512], dt, tag="acc")
                    slot = zslots[ci % 3]
                    for g in range(8):
                        pt = ptpool.tile([P, 8, 512], f16, tag="pt")
                        nc.vector.tensor_tensor(
                            pt[:, :, :cw],
                            slot[:, g, :cw].unsqueeze(1)
                            .broadcast_to((P, 8, cw)),
                            H32[:, :, c0:c0 + cw], OP.mult)
                        for b in range(8):
                            nc.tensor.matmul(
                                acc[:, :cw], W2P[:, 8 * g + b, :],
                                pt[:, b, :cw],
                                start=(g == 0 and b == 0), stop=False)
                        # PE gap fillers: two deferred ops per g slot
                        if deferred:
                            deferred.pop(0)()
                        if deferred:
                            deferred.pop(0)()
                    for b in range(8):
                        nc.tensor.matmul(acc[:, :cw], w2b2[:, b, :],
                                         H32[0:16, b, c0:c0 + cw],
                                         start=False, stop=(b == 7))
                    nc.scalar.copy(msgT16[:, c0:c0 + cw], acc[:, :cw])
                    for t in range(t0_, t0_ + nt_):
                        deferred.append(lambda t=t: msg_transpose(t))
                    deferred += [lambda ng=ng: scat_ng(ng) for ng in ready[ci]]
                    deferred += [lambda gg=gg: gru_group(gg)
                                 for gg in gru_ready[ci]]
                for fn in deferred:
                    fn()

    nc.compile()
    return nc


_CACHE = {}


def _get_program(CAP, W_band, base, contrib):
    key = (CAP, W_band, base, contrib)
    if key not in _CACHE:
        _CACHE[key] = _build_program(CAP, W_band, base, contrib)
    return _CACHE[key]


def kernel(h, edge_index, edge_features, W1, b1, W2, b2, W_ih, W_hh, b_ih, b_hh):
    from concourse import bass_utils

    in_maps, CAP, W_band, base, contrib = _host_prep(
        h, edge_index, edge_features, W1, b1, W2, b2, W_ih, W_hh, b_ih, b_hh)
    nc = _get_program(CAP, W_band, base, contrib)
    res = bass_utils.run_bass_kernel_spmd(nc, in_maps, core_ids=list(range(NCORES)))
    out = np.empty((N, H), np.float32)
    for c in range(NCORES):
        out[c * NS:(c + 1) * NS] = res.results[c]["out_hT"].T
    return out


# revision 53
# speedup vs baseline: 1.2494x; 1.2494x over previous
"""EdgeNetworkLayer Trainium2 kernel: 8-core SPMD, edges sharded BY TARGET.

Core c owns nodes [c*1024, (c+1)*1024) and every edge pointing into them, so
the per-shard segment_sum is complete locally and NO collective is needed;
each core runs the GRU on its own node shard and returns it.

messages[e,i] = sum_{k,j} z[e,k] * h_w[e,j] * W2[k, i*128+j]
with z = relu(ef @ W1 + b1); the bilinear form is one PE matmul chain with
contraction dim (k,j) = 64*128 = 8192:
  msgT[i, e] = sum_t W2p_t[p, i].T @ PT_t[p, e]
where tile t = (g, b), partition p = (a, c), k = 4g+a, j = 32b+c,
PT_t[p, e] = z[e, 4g+a] * h_w[e, 32b+c] (built on DVE in fp16, the pacing
engine). The h side (h_w gather, transpose, x4 block replication) is pure
host-known data movement and is shipped as inputs (hwT, H32) like the
one-hot scatter matrix; z is computed on device and DMA-replicated x32 per
chunk from a DRAM bounce (SBUF sources can't partition-broadcast), one op
per (chunk, a) across four rings with a 2-chunk lead.

Edges are processed in 512-column chunks; scatter + GRU + msg transposes are
interleaved one-op-per-g into the main loop's PE gaps so the PE stays
continuously busy (full p-state) while DVE paces.

Scatter: per node tile, PSUM-chained matmuls with stationary = msg tile
(fp16, PE transpose of msgT) and moving = band-limited one-hot S (exact 0/1
fp16), producing mT [i, node] directly. Z and GRU matmuls in float32r.
"""
import numpy as np

N, H, E, ED, MLP_HID = 8192, 128, 16384, 16, 64
NCORES = 8
P = 128
NS = N // NCORES          # 1024 nodes per core
NT = NS // P              # 8 node tiles per core
KG = 16                   # k-groups of 4
GRU_COLS = 256            # GRU column-group width (>=256 for f32r fast path)


def _host_prep(h, edge_index, edge_features, W1, b1, W2, b2, W_ih, W_hh, b_ih, b_hh):
    f32, f16 = np.float32, np.float16
    h = np.ascontiguousarray(h, f32)
    src_all = np.asarray(edge_index[0], np.int64)
    tgt_all = np.asarray(edge_index[1], np.int64)
    ef_all = np.asarray(edge_features, f32)

    order = np.argsort(tgt_all, kind="stable")
    s_s, t_s, ef_s = src_all[order], tgt_all[order], ef_all[order]
    shard_of = t_s // NS
    shards = []
    for c in range(NCORES):
        m = shard_of == c
        shards.append((s_s[m], t_s[m] - c * NS, ef_s[m]))
    CAP = ((max(len(s[0]) for s in shards) + P - 1) // P) * P
    ET = CAP // P

    # tile band plan, uniform across cores
    base = np.zeros(ET, np.int64)
    endv = np.zeros(ET, np.int64)
    any_real = np.zeros(ET, bool)
    for ti in range(ET):
        lo, hi = NS, 0
        for c in range(NCORES):
            seg = shards[c][1][ti * P:(ti + 1) * P]
            if len(seg):
                any_real[ti] = True
                lo = min(lo, int(seg.min()))
                hi = max(hi, int(seg.max()) + 1)
        if any_real[ti]:
            base[ti] = (lo // P) * P
            endv[ti] = hi
    W_band = P
    for ti in range(ET):
        if any_real[ti]:
            W_band = max(W_band, int(-((base[ti] - endv[ti]) // P)) * P)

    # contrib[ng] = edge tiles feeding node tile ng (union over cores)
    contrib = [[] for _ in range(NT)]
    for ti in range(ET):
        if not any_real[ti]:
            continue
        ngs = set()
        for c in range(NCORES):
            seg = shards[c][1][ti * P:(ti + 1) * P]
            if len(seg):
                ngs |= set(int(x) for x in np.unique(seg // P))
        for ng in sorted(ngs):
            contrib[ng].append(ti)

    hf16 = h.astype(f16)
    # (a=8, c=16) factorization: k = 8*gg + a, j = 16*bb + c, p = 16a + c
    W2r = np.asarray(W2, f32).reshape(MLP_HID, H, H)            # [k, i, j]
    W2g = W2r.reshape(8, 8, H, 8, 16)                           # [gg, a, i, bb, c]
    W2p = W2g.transpose(0, 3, 1, 4, 2).reshape(64, P, H)        # [(gg,bb), (a,c), i]
    W2P_host = np.ascontiguousarray(W2p.transpose(1, 0, 2).astype(f16))  # [p, 64, i]
    # [c, bb, i]: b2 tile pre-split for 16-row matmuls against H16 bb-slices
    W2b2_host = np.ascontiguousarray(
        np.asarray(b2, f32).reshape(H, 8, 16).transpose(2, 1, 0).astype(f16))
    W1p = np.concatenate([np.asarray(W1, f32), np.asarray(b1, f32)[None, :]], 0)

    W_ihT = np.ascontiguousarray(np.asarray(W_ih, f32).T)       # [128, 384]
    W_hhT = np.ascontiguousarray(np.asarray(W_hh, f32).T)
    b_ih = np.asarray(b_ih, f32)
    b_hh = np.asarray(b_hh, f32)
    b_r = (b_ih[:H] + b_hh[:H]).reshape(H, 1).astype(f32)
    b_z = (b_ih[H:2 * H] + b_hh[H:2 * H]).reshape(H, 1).astype(f32)
    b_in = b_ih[2 * H:].reshape(H, 1).astype(f32)
    b_hn = b_hh[2 * H:].reshape(H, 1).astype(f32)

    in_maps = []
    for c in range(NCORES):
        s, toff, ef = shards[c]
        n = len(s)
        s_pad = np.zeros(CAP, np.int64)
        s_pad[:n] = s
        ef_pad = np.zeros((CAP, ED), f32)
        ef_pad[:n] = ef
        efT = np.concatenate([ef_pad.T, np.ones((1, CAP), f32)], 0)   # [17, CAP]
        hwT = np.ascontiguousarray(hf16[s_pad].T)                     # [128, CAP]
        H32 = np.empty((P, 8, CAP), f16)
        for a in range(8):
            for b in range(8):
                H32[16 * a:16 * a + 16, b, :] = hwT[16 * b:16 * b + 16, :]
        Sband = np.zeros((P, ET, W_band), f16)
        idx = np.arange(n)
        Sband[idx % P, idx // P, toff - base[idx // P]] = 1.0
        hTs = np.ascontiguousarray(h[c * NS:(c + 1) * NS].T)          # [128, 1024]
        in_maps.append(dict(
            efT=efT, H32=H32, Sband=Sband, W2P=W2P_host,
            W2b2=W2b2_host, W1p=W1p, WihT=W_ihT, WhhT=W_hhT, b_r=b_r,
            b_z=b_z, b_in=b_in, b_hn=b_hn, hTs=hTs))
    return (in_maps, CAP, W_band, tuple(int(b) for b in base),
            tuple(tuple(cc) for cc in contrib))


def _build_program(CAP, W_band, base_arr, contrib):
    import concourse.bass as bass
    import concourse.bacc as bacc
    import concourse.tile as tile
    import concourse.mybir as mybir
    from concourse.masks import make_identity

    dt = mybir.dt.float32
    f16 = mybir.dt.float16
    f32r = mybir.dt.float32r
    AF = mybir.ActivationFunctionType
    OP = mybir.AluOpType

    ET = CAP // P
    chunks = []
    t0 = 0
    while t0 < ET:
        nt_ = min(4, ET - t0)
        chunks.append((t0, nt_))
        t0 += nt_
    NCH = len(chunks)
    last_tile_of_chunk = [t0_ + nt_ - 1 for (t0_, nt_) in chunks]

    ready = [[] for _ in range(NCH)]
    empty_ng = []
    for ng in range(NT):
        if not contrib[ng]:
            empty_ng.append(ng)
            continue
        need = max(contrib[ng])
        for ci in range(NCH):
            if need <= last_tile_of_chunk[ci]:
                ready[ci].append(ng)
                break
    ngrp = GRU_COLS // P
    ng_done_at = {ng: 0 for ng in empty_ng}
    for ci in range(NCH):
        for ng in ready[ci]:
            ng_done_at[ng] = ci
    gru_ready = [[] for _ in range(NCH)]
    for gg in range(NS // GRU_COLS):
        ci = max(ng_done_at[gg * ngrp + i] for i in range(ngrp))
        gru_ready[ci].append(gg)

    nc = bacc.Bacc("TRN2", target_bir_lowering=False, debug=False,
                   num_devices=NCORES)

    efT_d = nc.dram_tensor("efT", [ED + 1, CAP], f32r, kind="ExternalInput")
    H32_d = nc.dram_tensor("H32", [P, 8, CAP], f16, kind="ExternalInput")
    S_d = nc.dram_tensor("Sband", [P, ET, W_band], f16, kind="ExternalInput")
    W2P_d = nc.dram_tensor("W2P", [P, 64, H], f16, kind="ExternalInput")
    W2b2_d = nc.dram_tensor("W2b2", [16, 8, H], f16, kind="ExternalInput")
    W1p_d = nc.dram_tensor("W1p", [ED + 1, MLP_HID], f32r, kind="ExternalInput")
    WihT_d = nc.dram_tensor("WihT", [H, 3 * H], f32r, kind="ExternalInput")
    WhhT_d = nc.dram_tensor("WhhT", [H, 3 * H], f32r, kind="ExternalInput")
    br_d = nc.dram_tensor("b_r", [H, 1], dt, kind="ExternalInput")
    bz_d = nc.dram_tensor("b_z", [H, 1], dt, kind="ExternalInput")
    bin_d = nc.dram_tensor("b_in", [H, 1], dt, kind="ExternalInput")
    bhn_d = nc.dram_tensor("b_hn", [H, 1], dt, kind="ExternalInput")
    hTs_d = nc.dram_tensor("hTs", [H, NS], f32r, kind="ExternalInput")
    out_d = nc.dram_tensor("out_hT", [H, NS], dt, kind="ExternalOutput")

    with tile.TileContext(nc) as tc:
        with (
            tc.tile_pool(name="const", bufs=1) as cp,
            tc.tile_pool(name="dram", bufs=1, space="DRAM") as dram,
            tc.tile_pool(name="work", bufs=1) as wp,
        ):
            # ---------- startup
            ident16 = cp.tile([P, P], f16)
            make_identity(nc, ident16[:])
            efT = cp.tile([ED + 1, CAP], f32r)
            W1p = cp.tile([ED + 1, MLP_HID], f32r)
            nc.sync.dma_start(W1p[:], W1p_d[:])
            for (t0_, nt_) in chunks:
                c0, cw = t0_ * P, nt_ * P
                nc.sync.dma_start(efT[:, c0:c0 + cw], efT_d[:, c0:c0 + cw])

            H32 = cp.tile([P, 8, CAP], f16)
            c0w = chunks[0][1] * P
            nc.gpsimd.dma_start(H32[:, :, :c0w], H32_d[:, :, :c0w])

            W2P = cp.tile([P, 64, H], f16)
            w2b2 = cp.tile([16, 8, H], f16)
            WihT = cp.tile([H, 3 * H], f32r)
            WhhT = cp.tile([H, 3 * H], f32r)
            b_r = cp.tile([H, 1], dt)
            b_z = cp.tile([H, 1], dt)
            b_in = cp.tile([H, 1], dt)
            b_hn = cp.tile([H, 1], dt)
            hTs = cp.tile([H, NS], f32r)
            Sband = cp.tile([P, ET, W_band], f16)

            zT = wp.tile([MLP_HID, CAP], f16)
            zT_dram = dram.tile([MLP_HID, CAP], f16)

            # Z: zT = relu(W1p.T @ efT), chunked, bounced to DRAM
            with tc.tile_pool(name="psz", bufs=2, space="PSUM") as psz:
                zc0 = 0
                while zc0 < CAP:
                    cw = min(512, CAP - zc0)
                    zps = psz.tile([MLP_HID, 512], dt, tag="zps")
                    nc.tensor.matmul(zps[:, :cw], W1p[:],
                                     efT[:, zc0:zc0 + cw],
                                     start=True, stop=True)
                    nc.scalar.activation(zT[:, zc0:zc0 + cw], zps[:, :cw],
                                         AF.Relu)
                    nc.scalar.dma_start(zT_dram[:, zc0:zc0 + cw],
                                        zT[:, zc0:zc0 + cw])
                    zc0 += cw

            def emit_z32(ci, slot, engines, fine=False):
                (t0_, nt_) = chunks[ci]
                c0, cw = t0_ * P, nt_ * P
                zv = zT_dram[:, c0:c0 + cw].rearrange("(g a) e -> a g e", a=8)
                if fine:
                    k = 0
                    for gp in range(4):
                        for a in range(8):
                            eng = engines[k % len(engines)]
                            k += 1
                            eng.dma_start(
                                slot[16 * a:16 * a + 16,
                                     2 * gp:2 * gp + 2, :cw],
                                zv[a:a + 1, 2 * gp:2 * gp + 2]
                                .broadcast_to((16, 2, cw)))
                else:
                    for a in range(8):
                        eng = engines[a % len(engines)]
                        eng.dma_start(
                            slot[16 * a:16 * a + 16, :, :cw],
                            zv[a:a + 1].broadcast_to((16, 8, cw)))

            # ---------- main pipeline
            msgT16 = wp.tile([P, CAP], f16)
            msg = wp.tile([P, ET, P], f16)
            mT = wp.tile([H, NS], f32r)
            out_sb = wp.tile([H, NS], dt)
            for ng in empty_ng:
                nc.gpsimd.memset(mT[:, ng * P:(ng + 1) * P], 0.0)

            with (
                tc.tile_pool(name="psacc", bufs=2, space="PSUM") as psacc,
                tc.tile_pool(name="pstr", bufs=2, space="PSUM") as pstr,
                tc.tile_pool(name="psm", bufs=2, space="PSUM") as psm,
                tc.tile_pool(name="psg", bufs=1, space="PSUM") as psg,
                tc.tile_pool(name="zpool", bufs=1) as zpool,
                tc.tile_pool(name="ptpool", bufs=3) as ptpool,
            ):
                def scat_ng(ng):
                    pm = psm.tile([P, P], dt, tag="pm")
                    for idx, ti in enumerate(contrib[ng]):
                        off = ng * P - int(base_arr[ti])
                        nc.tensor.matmul(
                            pm[:], msg[:, ti, :], Sband[:, ti, off:off + P],
                            start=(idx == 0),
                            stop=(idx == len(contrib[ng]) - 1))
                    nc.scalar.copy(mT[:, ng * P:(ng + 1) * P], pm[:])

                def msg_transpose(t):
                    tp = pstr.tile([P, P], f16, tag="tp")
                    nc.tensor.transpose(
                        tp[:], msgT16[:, t * P:(t + 1) * P], ident16[:])
                    nc.scalar.copy(msg[:, t, :], tp[:])

                def gru_group(gg):
                    ew = nc.vector if gg >= (NS // GRU_COLS) - 2 else nc.gpsimd
                    c0 = gg * GRU_COLS
                    csl = slice(c0, c0 + GRU_COLS)
                    rz_ps = psg.tile([H, 2, GRU_COLS], dt, tag="rzp")
                    nn_ps = psg.tile([H, 2, GRU_COLS], dt, tag="nnp")
                    gin_ps = nn_ps[:, 0, :]
                    ghn_ps = nn_ps[:, 1, :]
                    for q in range(2):
                        nc.tensor.matmul(rz_ps[:, q, :],
                                         WihT[:, q * H:(q + 1) * H],
                                         mT[:, csl], start=True, stop=False)
                        nc.tensor.matmul(rz_ps[:, q, :],
                                         WhhT[:, q * H:(q + 1) * H],
                                         hTs[:, csl], start=False, stop=True)
                    nc.tensor.matmul(gin_ps, WihT[:, 2 * H:3 * H],
                                     mT[:, csl], start=True, stop=True)
                    nc.tensor.matmul(ghn_ps, WhhT[:, 2 * H:3 * H],
                                     hTs[:, csl], start=True, stop=True)
                    rz = wp.tile([H, 2, GRU_COLS], dt, tag=f"rz{gg % 2}")
                    nc.scalar.activation(rz[:, 0, :], rz_ps[:, 0, :],
                                         AF.Sigmoid, bias=b_r[:])
                    nc.scalar.activation(rz[:, 1, :], rz_ps[:, 1, :],
                                         AF.Sigmoid, bias=b_z[:])
                    ghn = wp.tile([H, GRU_COLS], dt, tag=f"ghn{gg % 2}")
                    nc.scalar.activation(ghn[:], ghn_ps, AF.Identity,
                                         bias=b_hn[:])
                    ew.tensor_mul(ghn[:], rz[:, 0, :], ghn[:])
                    nc.vector.tensor_add(ghn[:], ghn[:], gin_ps)
                    ng_ = wp.tile([H, GRU_COLS], dt, tag=f"ng{gg % 2}")
                    nc.scalar.activation(ng_[:], ghn[:], AF.Tanh, bias=b_in[:])
                    dif = wp.tile([H, GRU_COLS], dt, tag=f"dif{gg % 2}")
                    ew.tensor_sub(dif[:], hTs[:, csl].bitcast(dt), ng_[:])
                    ew.tensor_mul(dif[:], rz[:, 1, :], dif[:])
                    ew.tensor_add(out_sb[:, csl], ng_[:], dif[:])
                    nc.sync.dma_start(out_d[:, csl], out_sb[:, csl])

                # startup tail: chunk-0/1 z32 first on each ring (FIFO),
                # then bulk loads behind them.
                zslot0 = zpool.tile([P, 8, 512], f16, tag="zs0")
                zslot1 = zpool.tile([P, 8, 512], f16, tag="zs1")
                zslot2 = zpool.tile([P, 8, 512], f16, tag="zs2")
                zslots = [zslot0, zslot1, zslot2]
                emit_z32(0, zslots[0], [nc.sync, nc.scalar, nc.gpsimd],
                         fine=True)

                # Sequencing: engine queues issue in order, so tiny token
                # DMAs (reading a just-landed z32 region, f16-matched) hold
                # bulk loads behind the chunk-0/1 critical z32 stream.
                def tok_dma(eng, dst, slot_i, g_idx):
                    eng.dma_start(dst, zslots[slot_i][96:97, g_idx, 0:1])

                nc.sync.dma_start(W2P[:, 0:16, :], W2P_d[:, 0:16, :])
                tok_dma(nc.sync, W2P[0:1, 16, 0:1], 0, 3)
                nc.sync.dma_start(W2P[:, 16:32, :], W2P_d[:, 16:32, :])
                tok_dma(nc.sync, W2P[0:1, 32, 0:1], 0, 5)
                nc.sync.dma_start(W2P[:, 32:48, :], W2P_d[:, 32:48, :])
                tok_dma(nc.sync, W2P[0:1, 48, 0:1], 0, 7)
                nc.sync.dma_start(W2P[:, 48:64, :], W2P_d[:, 48:64, :])
                if NCH > 1:
                    emit_z32(1, zslots[1], [nc.sync, nc.scalar, nc.gpsimd],
                             fine=True)
                t00 = chunks[0][0]
                tok_dma(nc.sync, Sband[0:1, t00, 0:1], 1 if NCH > 1 else 0, 1)
                nc.sync.dma_start(
                    Sband[:, t00:t00 + chunks[0][1], :],
                    S_d[:, t00:t00 + chunks[0][1], :])
                nc.sync.dma_start(w2b2[:], W2b2_d[:])
                nc.sync.dma_start(WihT[:], WihT_d[:])
                nc.sync.dma_start(WhhT[:], WhhT_d[:])
                nc.sync.dma_start(hTs[:], hTs_d[:])
                nc.sync.dma_start(b_r[:], br_d[:])
                nc.sync.dma_start(b_z[:], bz_d[:])
                nc.sync.dma_start(b_in[:], bin_d[:])
                nc.sync.dma_start(b_hn[:], bhn_d[:])
                # H32 rest, gated + progressive, on the gpsimd queue
                tok_dma(nc.gpsimd, H32[0:1, 0, c0w:c0w + 1], 0, 5)
                if CAP > 1024 and NCH > 2:
                    nc.gpsimd.dma_start(H32[:, :, c0w:1024],
                                        H32_d[:, :, c0w:1024])
                    prev = 1024
                    for ci_ in range(2, NCH):
                        cend = min((chunks[ci_][0] + chunks[ci_][1]) * P, CAP)
                        tok_dma(nc.gpsimd, H32[0:1, 0, prev:prev + 1],
                                1, 2 * ci_ - 3 if 2 * ci_ - 3 < 8 else 7)
                        nc.gpsimd.dma_start(H32[:, :, prev:cend],
                                            H32_d[:, :, prev:cend])
                        prev = cend
                else:
                    nc.gpsimd.dma_start(H32[:, :, c0w:], H32_d[:, :, c0w:])

                deferred = []
                for ci, (t0_, nt_) in enumerate(chunks):
                    c0, cw = t0_ * P, nt_ * P
                    if ci + 2 < NCH:
                        emit_z32(ci + 2, zslots[(ci + 2) % 3],
                                 [nc.sync, nc.scalar])
                    if ci + 1 < NCH:
                        t1_, n1_ = chunks[ci + 1]
                        nc.sync.dma_start(Sband[:, t1_:t1_ + n1_, :],
                                          S_d[:, t1_:t1_ + n1_, :])
                    acc = psacc.tile([P, 512], dt, tag="acc")
                    slot = zslots[ci % 3]
                    for g in range(8):
                        pt = ptpool.tile([P, 8, 512], f16, tag="pt")
                        nc.vector.tensor_tensor(
                            pt[:, :, :cw],
                            slot[:, g, :cw].unsqueeze(1)
                            .broadcast_to((P, 8, cw)),
                            H32[:, :, c0:c0 + cw], OP.mult)
                        for b in range(8):
                            nc.tensor.matmul(
                                acc[:, :cw], W2P[:, 8 * g + b, :],
                                pt[:, b, :cw],
                                start=(g == 0 and b == 0), stop=False)
                        # PE gap fillers: two deferred ops per g slot
                        if deferred:
                            deferred.pop(0)()
                        if deferred:
                            deferred.pop(0)()
                    for b in range(8):
                        nc.tensor.matmul(acc[:, :cw], w2b2[:, b, :],
                                         H32[0:16, b, c0:c0 + cw],
                                         start=False, stop=(b == 7))
                    nc.scalar.copy(msgT16[:, c0:c0 + cw], acc[:, :cw])
                    for t in range(t0_, t0_ + nt_):
                        deferred.append(lambda t=t: msg_transpose(t))
                    deferred += [lambda ng=ng: scat_ng(ng) for ng in ready[ci]]
                    deferred += [lambda gg=gg: gru_group(gg)
                                 for gg in gru_ready[ci]]
                for fn in deferred:
                    fn()

    nc.compile()
    return nc


_CACHE = {}


def _get_program(CAP, W_band, base, contrib):
    key = (CAP, W_band, base, contrib)
    if key not in _CACHE:
        _CACHE[key] = _build_program(CAP, W_band, base, contrib)
    return _CACHE[key]


def kernel(h, edge_index, edge_features, W1, b1, W2, b2, W_ih, W_hh, b_ih, b_hh):
    from concourse import bass_utils

    in_maps, CAP, W_band, base, contrib = _host_prep(
        h, edge_index, edge_features, W1, b1, W2, b2, W_ih, W_hh, b_ih, b_hh)
    nc = _get_program(CAP, W_band, base, contrib)
    res = bass_utils.run_bass_kernel_spmd(nc, in_maps, core_ids=list(range(NCORES)))
    out = np.empty((N, H), np.float32)
    for c in range(NCORES):
        out[c * NS:(c + 1) * NS] = res.results[c]["out_hT"].T
    return out


# revision 56
# speedup vs baseline: 1.2558x; 1.0051x over previous
"""EdgeNetworkLayer Trainium2 kernel: 8-core SPMD, edges sharded BY TARGET.

Core c owns nodes [c*1024, (c+1)*1024) and every edge pointing into them, so
the per-shard segment_sum is complete locally and NO collective is needed;
each core runs the GRU on its own node shard and returns it.

messages[e,i] = sum_{k,j} z[e,k] * h_w[e,j] * W2[k, i*128+j]
with z = relu(ef @ W1 + b1); the bilinear form is one PE matmul chain with
contraction dim (k,j) = 64*128 = 8192:
  msgT[i, e] = sum_t W2p_t[p, i].T @ PT_t[p, e]
where tile t = (g, b), partition p = (a, c), k = 4g+a, j = 32b+c,
PT_t[p, e] = z[e, 4g+a] * h_w[e, 32b+c] (built on DVE in fp16, the pacing
engine). The h side (h_w gather, transpose, x4 block replication) is pure
host-known data movement and is shipped as inputs (hwT, H32) like the
one-hot scatter matrix; z is computed on device and DMA-replicated x32 per
chunk from a DRAM bounce (SBUF sources can't partition-broadcast), one op
per (chunk, a) across four rings with a 2-chunk lead.

Edges are processed in 512-column chunks; scatter + GRU + msg transposes are
interleaved one-op-per-g into the main loop's PE gaps so the PE stays
continuously busy (full p-state) while DVE paces.

Scatter: per node tile, PSUM-chained matmuls with stationary = msg tile
(fp16, PE transpose of msgT) and moving = band-limited one-hot S (exact 0/1
fp16), producing mT [i, node] directly. Z and GRU matmuls in float32r.
"""
import numpy as np

N, H, E, ED, MLP_HID = 8192, 128, 16384, 16, 64
NCORES = 8
P = 128
NS = N // NCORES          # 1024 nodes per core
NT = NS // P              # 8 node tiles per core
KG = 16                   # k-groups of 4
GRU_COLS = 256            # GRU column-group width (>=256 for f32r fast path)


def _host_prep(h, edge_index, edge_features, W1, b1, W2, b2, W_ih, W_hh, b_ih, b_hh):
    f32, f16 = np.float32, np.float16
    h = np.ascontiguousarray(h, f32)
    src_all = np.asarray(edge_index[0], np.int64)
    tgt_all = np.asarray(edge_index[1], np.int64)
    ef_all = np.asarray(edge_features, f32)

    order = np.argsort(tgt_all, kind="stable")
    s_s, t_s, ef_s = src_all[order], tgt_all[order], ef_all[order]
    shard_of = t_s // NS
    shards = []
    for c in range(NCORES):
        m = shard_of == c
        shards.append((s_s[m], t_s[m] - c * NS, ef_s[m]))
    CAP = ((max(len(s[0]) for s in shards) + P - 1) // P) * P
    ET = CAP // P

    # tile band plan, uniform across cores
    base = np.zeros(ET, np.int64)
    endv = np.zeros(ET, np.int64)
    any_real = np.zeros(ET, bool)
    for ti in range(ET):
        lo, hi = NS, 0
        for c in range(NCORES):
            seg = shards[c][1][ti * P:(ti + 1) * P]
            if len(seg):
                any_real[ti] = True
                lo = min(lo, int(seg.min()))
                hi = max(hi, int(seg.max()) + 1)
        if any_real[ti]:
            base[ti] = (lo // P) * P
            endv[ti] = hi
    W_band = P
    for ti in range(ET):
        if any_real[ti]:
            W_band = max(W_band, int(-((base[ti] - endv[ti]) // P)) * P)

    # contrib[ng] = edge tiles feeding node tile ng (union over cores)
    contrib = [[] for _ in range(NT)]
    for ti in range(ET):
        if not any_real[ti]:
            continue
        ngs = set()
        for c in range(NCORES):
            seg = shards[c][1][ti * P:(ti + 1) * P]
            if len(seg):
                ngs |= set(int(x) for x in np.unique(seg // P))
        for ng in sorted(ngs):
            contrib[ng].append(ti)

    hf16 = h.astype(f16)
    # (a=8, c=16) factorization: k = 8*gg + a, j = 16*bb + c, p = 16a + c
    W2r = np.asarray(W2, f32).reshape(MLP_HID, H, H)            # [k, i, j]
    W2g = W2r.reshape(8, 8, H, 8, 16)                           # [gg, a, i, bb, c]
    W2p = W2g.transpose(0, 3, 1, 4, 2).reshape(64, P, H)        # [(gg,bb), (a,c), i]
    W2P_host = np.ascontiguousarray(W2p.transpose(1, 0, 2).astype(f16))  # [p, 64, i]
    # [c, bb, i]: b2 tile pre-split for 16-row matmuls against H16 bb-slices
    W2b2_host = np.ascontiguousarray(
        np.asarray(b2, f32).reshape(H, 8, 16).transpose(2, 1, 0).astype(f16))
    W1p = np.concatenate([np.asarray(W1, f32), np.asarray(b1, f32)[None, :]], 0)

    W_ihT = np.ascontiguousarray(np.asarray(W_ih, f32).T)       # [128, 384]
    W_hhT = np.ascontiguousarray(np.asarray(W_hh, f32).T)
    b_ih = np.asarray(b_ih, f32)
    b_hh = np.asarray(b_hh, f32)
    b_r = (b_ih[:H] + b_hh[:H]).reshape(H, 1).astype(f32)
    b_z = (b_ih[H:2 * H] + b_hh[H:2 * H]).reshape(H, 1).astype(f32)
    b_in = b_ih[2 * H:].reshape(H, 1).astype(f32)
    b_hn = b_hh[2 * H:].reshape(H, 1).astype(f32)

    in_maps = []
    for c in range(NCORES):
        s, toff, ef = shards[c]
        n = len(s)
        s_pad = np.zeros(CAP, np.int64)
        s_pad[:n] = s
        ef_pad = np.zeros((CAP, ED), f32)
        ef_pad[:n] = ef
        efT = np.concatenate([ef_pad.T, np.ones((1, CAP), f32)], 0)   # [17, CAP]
        hwT = np.ascontiguousarray(hf16[s_pad].T)                     # [128, CAP]
        H32 = np.empty((P, 8, CAP), f16)
        for a in range(8):
            for b in range(8):
                H32[16 * a:16 * a + 16, b, :] = hwT[16 * b:16 * b + 16, :]
        Sband = np.zeros((P, ET, W_band), f16)
        idx = np.arange(n)
        Sband[idx % P, idx // P, toff - base[idx // P]] = 1.0
        hTs = np.ascontiguousarray(h[c * NS:(c + 1) * NS].T)          # [128, 1024]
        in_maps.append(dict(
            efT=efT, H32=H32, Sband=Sband, W2P=W2P_host,
            W2b2=W2b2_host, W1p=W1p, WihT=W_ihT, WhhT=W_hhT, b_r=b_r,
            b_z=b_z, b_in=b_in, b_hn=b_hn, hTs=hTs))
    return (in_maps, CAP, W_band, tuple(int(b) for b in base),
            tuple(tuple(cc) for cc in contrib))


def _build_program(CAP, W_band, base_arr, contrib):
    import concourse.bass as bass
    import concourse.bacc as bacc
    import concourse.tile as tile
    import concourse.mybir as mybir
    from concourse.masks import make_identity

    dt = mybir.dt.float32
    f16 = mybir.dt.float16
    f32r = mybir.dt.float32r
    AF = mybir.ActivationFunctionType
    OP = mybir.AluOpType

    ET = CAP // P
    chunks = []
    t0 = 0
    while t0 < ET:
        nt_ = min(4, ET - t0)
        chunks.append((t0, nt_))
        t0 += nt_
    NCH = len(chunks)
    last_tile_of_chunk = [t0_ + nt_ - 1 for (t0_, nt_) in chunks]

    ready = [[] for _ in range(NCH)]
    empty_ng = []
    for ng in range(NT):
        if not contrib[ng]:
            empty_ng.append(ng)
            continue
        need = max(contrib[ng])
        for ci in range(NCH):
            if need <= last_tile_of_chunk[ci]:
                ready[ci].append(ng)
                break
    ngrp = GRU_COLS // P
    ng_done_at = {ng: 0 for ng in empty_ng}
    for ci in range(NCH):
        for ng in ready[ci]:
            ng_done_at[ng] = ci
    gru_ready = [[] for _ in range(NCH)]
    for gg in range(NS // GRU_COLS):
        ci = max(ng_done_at[gg * ngrp + i] for i in range(ngrp))
        gru_ready[ci].append(gg)

    nc = bacc.Bacc("TRN2", target_bir_lowering=False, debug=False,
                   num_devices=NCORES)

    efT_d = nc.dram_tensor("efT", [ED + 1, CAP], f32r, kind="ExternalInput")
    H32_d = nc.dram_tensor("H32", [P, 8, CAP], f16, kind="ExternalInput")
    S_d = nc.dram_tensor("Sband", [P, ET, W_band], f16, kind="ExternalInput")
    W2P_d = nc.dram_tensor("W2P", [P, 64, H], f16, kind="ExternalInput")
    W2b2_d = nc.dram_tensor("W2b2", [16, 8, H], f16, kind="ExternalInput")
    W1p_d = nc.dram_tensor("W1p", [ED + 1, MLP_HID], f32r, kind="ExternalInput")
    WihT_d = nc.dram_tensor("WihT", [H, 3 * H], f32r, kind="ExternalInput")
    WhhT_d = nc.dram_tensor("WhhT", [H, 3 * H], f32r, kind="ExternalInput")
    br_d = nc.dram_tensor("b_r", [H, 1], dt, kind="ExternalInput")
    bz_d = nc.dram_tensor("b_z", [H, 1], dt, kind="ExternalInput")
    bin_d = nc.dram_tensor("b_in", [H, 1], dt, kind="ExternalInput")
    bhn_d = nc.dram_tensor("b_hn", [H, 1], dt, kind="ExternalInput")
    hTs_d = nc.dram_tensor("hTs", [H, NS], f32r, kind="ExternalInput")
    out_d = nc.dram_tensor("out_hT", [H, NS], dt, kind="ExternalOutput")

    with tile.TileContext(nc) as tc:
        with (
            tc.tile_pool(name="const", bufs=1) as cp,
            tc.tile_pool(name="dram", bufs=1, space="DRAM") as dram,
            tc.tile_pool(name="work", bufs=1) as wp,
        ):
            # ---------- startup
            ident16 = cp.tile([P, P], f16)
            make_identity(nc, ident16[:])
            efT = cp.tile([ED + 1, CAP], f32r)
            W1p = cp.tile([ED + 1, MLP_HID], f32r)
            nc.sync.dma_start(W1p[:], W1p_d[:])
            for (t0_, nt_) in chunks:
                c0, cw = t0_ * P, nt_ * P
                nc.sync.dma_start(efT[:, c0:c0 + cw], efT_d[:, c0:c0 + cw])

            H32 = cp.tile([P, 8, CAP], f16)
            c0w = chunks[0][1] * P
            nc.gpsimd.dma_start(H32[:, :, :c0w], H32_d[:, :, :c0w])

            W2P = cp.tile([P, 64, H], f16)
            w2b2 = cp.tile([16, 8, H], f16)
            WihT = cp.tile([H, 3 * H], f32r)
            WhhT = cp.tile([H, 3 * H], f32r)
            b_r = cp.tile([H, 1], dt)
            b_z = cp.tile([H, 1], dt)
            b_in = cp.tile([H, 1], dt)
            b_hn = cp.tile([H, 1], dt)
            hTs = cp.tile([H, NS], f32r)
            Sband = cp.tile([P, ET, W_band], f16)

            zT = wp.tile([MLP_HID, CAP], f16)
            zT_dram = dram.tile([MLP_HID, CAP], f16)

            # Z: zT = relu(W1p.T @ efT), chunked, bounced to DRAM
            with tc.tile_pool(name="psz", bufs=2, space="PSUM") as psz:
                zc0 = 0
                while zc0 < CAP:
                    cw = min(512, CAP - zc0)
                    zps = psz.tile([MLP_HID, 512], dt, tag="zps")
                    nc.tensor.matmul(zps[:, :cw], W1p[:],
                                     efT[:, zc0:zc0 + cw],
                                     start=True, stop=True)
                    nc.scalar.activation(zT[:, zc0:zc0 + cw], zps[:, :cw],
                                         AF.Relu)
                    nc.scalar.dma_start(zT_dram[:, zc0:zc0 + cw],
                                        zT[:, zc0:zc0 + cw])
                    zc0 += cw

            def emit_z32(ci, slot, engines, fine=False):
                (t0_, nt_) = chunks[ci]
                c0, cw = t0_ * P, nt_ * P
                zv = zT_dram[:, c0:c0 + cw].rearrange("(g a) e -> a g e", a=8)
                if fine:
                    k = 0
                    for gp in range(4):
                        for a in range(8):
                            eng = engines[k % len(engines)]
                            k += 1
                            eng.dma_start(
                                slot[16 * a:16 * a + 16,
                                     2 * gp:2 * gp + 2, :cw],
                                zv[a:a + 1, 2 * gp:2 * gp + 2]
                                .broadcast_to((16, 2, cw)))
                else:
                    for a in range(8):
                        eng = engines[a % len(engines)]
                        eng.dma_start(
                            slot[16 * a:16 * a + 16, :, :cw],
                            zv[a:a + 1].broadcast_to((16, 8, cw)))

            # ---------- main pipeline
            msgT16 = wp.tile([P, CAP], f16)
            msg = wp.tile([P, ET, P], f16)
            mT = wp.tile([H, NS], f32r)
            out_sb = wp.tile([H, NS], dt)
            for ng in empty_ng:
                nc.gpsimd.memset(mT[:, ng * P:(ng + 1) * P], 0.0)

            with (
                tc.tile_pool(name="psacc", bufs=2, space="PSUM") as psacc,
                tc.tile_pool(name="pstr", bufs=2, space="PSUM") as pstr,
                tc.tile_pool(name="psm", bufs=2, space="PSUM") as psm,
                tc.tile_pool(name="psg", bufs=1, space="PSUM") as psg,
                tc.tile_pool(name="zpool", bufs=1) as zpool,
                tc.tile_pool(name="ptpool", bufs=3) as ptpool,
            ):
                def scat_ng(ng):
                    pm = psm.tile([P, P], dt, tag="pm")
                    for idx, ti in enumerate(contrib[ng]):
                        off = ng * P - int(base_arr[ti])
                        nc.tensor.matmul(
                            pm[:], msg[:, ti, :], Sband[:, ti, off:off + P],
                            start=(idx == 0),
                            stop=(idx == len(contrib[ng]) - 1))
                    nc.scalar.copy(mT[:, ng * P:(ng + 1) * P], pm[:])

                def msg_transpose(t):
                    tp = pstr.tile([P, P], f16, tag="tp")
                    nc.tensor.transpose(
                        tp[:], msgT16[:, t * P:(t + 1) * P], ident16[:])
                    nc.scalar.copy(msg[:, t, :], tp[:])

                def gru_group(gg):
                    ew = nc.vector if gg >= (NS // GRU_COLS) - 2 else nc.gpsimd
                    c0 = gg * GRU_COLS
                    csl = slice(c0, c0 + GRU_COLS)
                    rz_ps = psg.tile([H, 2, GRU_COLS], dt, tag="rzp")
                    nn_ps = psg.tile([H, 2, GRU_COLS], dt, tag="nnp")
                    gin_ps = nn_ps[:, 0, :]
                    ghn_ps = nn_ps[:, 1, :]
                    for q in range(2):
                        nc.tensor.matmul(rz_ps[:, q, :],
                                         WihT[:, q * H:(q + 1) * H],
                                         mT[:, csl], start=True, stop=False)
                        nc.tensor.matmul(rz_ps[:, q, :],
                                         WhhT[:, q * H:(q + 1) * H],
                                         hTs[:, csl], start=False, stop=True)
                    nc.tensor.matmul(gin_ps, WihT[:, 2 * H:3 * H],
                                     mT[:, csl], start=True, stop=True)
                    nc.tensor.matmul(ghn_ps, WhhT[:, 2 * H:3 * H],
                                     hTs[:, csl], start=True, stop=True)
                    rz = wp.tile([H, 2, GRU_COLS], dt, tag=f"rz{gg % 2}")
                    nc.scalar.activation(rz[:, 0, :], rz_ps[:, 0, :],
                                         AF.Sigmoid, bias=b_r[:])
                    nc.scalar.activation(rz[:, 1, :], rz_ps[:, 1, :],
                                         AF.Sigmoid, bias=b_z[:])
                    ghn = wp.tile([H, GRU_COLS], dt, tag=f"ghn{gg % 2}")
                    nc.scalar.activation(ghn[:], ghn_ps, AF.Identity,
                                         bias=b_hn[:])
                    ew.tensor_mul(ghn[:], rz[:, 0, :], ghn[:])
                    nc.vector.tensor_add(ghn[:], ghn[:], gin_ps)
                    ng_ = wp.tile([H, GRU_COLS], dt, tag=f"ng{gg % 2}")
                    nc.scalar.activation(ng_[:], ghn[:], AF.Tanh, bias=b_in[:])
                    dif = wp.tile([H, GRU_COLS], dt, tag=f"dif{gg % 2}")
                    ew.tensor_sub(dif[:], hTs[:, csl].bitcast(dt), ng_[:])
                    ew.tensor_mul(dif[:], rz[:, 1, :], dif[:])
                    ew.tensor_add(out_sb[:, csl], ng_[:], dif[:])
                    nc.sync.dma_start(out_d[:, csl], out_sb[:, csl])

                # startup tail: chunk-0/1 z32 first on each ring (FIFO),
                # then bulk loads behind them.
                zslot0 = zpool.tile([P, 8, 512], f16, tag="zs0")
                zslot1 = zpool.tile([P, 8, 512], f16, tag="zs1")
                zslot2 = zpool.tile([P, 8, 512], f16, tag="zs2")
                zslots = [zslot0, zslot1, zslot2]
                emit_z32(0, zslots[0], [nc.sync, nc.scalar, nc.gpsimd],
                         fine=True)

                # Sequencing: engine queues issue in order, so tiny token
                # DMAs (reading a just-landed z32 region, f16-matched) hold
                # bulk loads behind the chunk-0/1 critical z32 stream.
                def tok_dma(eng, dst, slot_i, g_idx):
                    eng.dma_start(dst, zslots[slot_i][96:97, g_idx, 0:1])

                nc.sync.dma_start(W2P[:, 0:16, :], W2P_d[:, 0:16, :])
                tok_dma(nc.sync, W2P[0:1, 16, 0:1], 0, 3)
                nc.sync.dma_start(W2P[:, 16:32, :], W2P_d[:, 16:32, :])
                tok_dma(nc.sync, W2P[0:1, 32, 0:1], 0, 5)
                nc.sync.dma_start(W2P[:, 32:48, :], W2P_d[:, 32:48, :])
                tok_dma(nc.sync, W2P[0:1, 48, 0:1], 0, 7)
                nc.sync.dma_start(W2P[:, 48:64, :], W2P_d[:, 48:64, :])
                if NCH > 1:
                    emit_z32(1, zslots[1], [nc.sync, nc.scalar, nc.gpsimd],
                             fine=True)
                t00 = chunks[0][0]
                tok_dma(nc.sync, Sband[0:1, t00, 0:1], 1 if NCH > 1 else 0, 1)
                nc.sync.dma_start(
                    Sband[:, t00:t00 + chunks[0][1], :],
                    S_d[:, t00:t00 + chunks[0][1], :])
                nc.sync.dma_start(w2b2[:], W2b2_d[:])
                nc.sync.dma_start(WihT[:], WihT_d[:])
                nc.sync.dma_start(WhhT[:], WhhT_d[:])
                nc.sync.dma_start(hTs[:], hTs_d[:])
                nc.sync.dma_start(b_r[:], br_d[:])
                nc.sync.dma_start(b_z[:], bz_d[:])
                nc.sync.dma_start(b_in[:], bin_d[:])
                nc.sync.dma_start(b_hn[:], bhn_d[:])
                # H32 rest, gated + progressive, on the gpsimd queue
                tok_dma(nc.gpsimd, H32[0:1, 0, c0w:c0w + 1], 0, 5)
                if CAP > 1024 and NCH > 2:
                    nc.gpsimd.dma_start(H32[:, :, c0w:1024],
                                        H32_d[:, :, c0w:1024])
                    prev = 1024
                    for ci_ in range(2, NCH):
                        cend = min((chunks[ci_][0] + chunks[ci_][1]) * P, CAP)
                        tok_dma(nc.gpsimd, H32[0:1, 0, prev:prev + 1],
                                1, 2 * ci_ - 3 if 2 * ci_ - 3 < 8 else 7)
                        nc.gpsimd.dma_start(H32[:, :, prev:cend],
                                            H32_d[:, :, prev:cend])
                        prev = cend
                else:
                    nc.gpsimd.dma_start(H32[:, :, c0w:], H32_d[:, :, c0w:])

                deferred = []
                for ci, (t0_, nt_) in enumerate(chunks):
                    c0, cw = t0_ * P, nt_ * P
                    if ci + 2 < NCH:
                        emit_z32(ci + 2, zslots[(ci + 2) % 3],
                                 [nc.sync, nc.scalar])
                    if ci + 1 < NCH:
                        t1_, n1_ = chunks[ci + 1]
                        nc.sync.dma_start(Sband[:, t1_:t1_ + n1_, :],
                                          S_d[:, t1_:t1_ + n1_, :])
                    acc = psacc.tile([P, 512], dt, tag="acc")
                    slot = zslots[ci % 3]
                    for g in range(8):
                        pt = ptpool.tile([P, 8, 512], f16, tag="pt")
                        nc.vector.tensor_tensor(
                            pt[:, :, :cw],
                            slot[:, g, :cw].unsqueeze(1)
                            .broadcast_to((P, 8, cw)),
                            H32[:, :, c0:c0 + cw], OP.mult)
                        for b in range(8):
                            nc.tensor.matmul(
                                acc[:, :cw], W2P[:, 8 * g + b, :],
                                pt[:, b, :cw],
                                start=(g == 0 and b == 0), stop=False)
                        # PE gap fillers: two deferred ops per g slot
                        if deferred:
                            deferred.pop(0)()
                        if deferred:
                            deferred.pop(0)()
                    for b in range(8):
                        nc.tensor.matmul(acc[:, :cw], w2b2[:, b, :],
                                         H32[0:16, b, c0:c0 + cw],
                                         start=False, stop=(b == 7))
                    nc.scalar.copy(msgT16[:, c0:c0 + cw], acc[:, :cw])
                    for t in range(t0_, t0_ + nt_):
                        deferred.append(lambda t=t: msg_transpose(t))
                    deferred += [lambda ng=ng: scat_ng(ng) for ng in ready[ci]]
                    deferred += [lambda gg=gg: gru_group(gg)
                                 for gg in gru_ready[ci]]
                for fn in deferred:
                    fn()

    nc.compile()
    return nc


_CACHE = {}


def _get_program(CAP, W_band, base, contrib):
    key = (CAP, W_band, base, contrib)
    if key not in _CACHE:
        _CACHE[key] = _build_program(CAP, W_band, base, contrib)
    return _CACHE[key]


def kernel(h, edge_index, edge_features, W1, b1, W2, b2, W_ih, W_hh, b_ih, b_hh):
    from concourse import bass_utils

    in_maps, CAP, W_band, base, contrib = _host_prep(
        h, edge_index, edge_features, W1, b1, W2, b2, W_ih, W_hh, b_ih, b_hh)
    nc = _get_program(CAP, W_band, base, contrib)
    res = bass_utils.run_bass_kernel_spmd(nc, in_maps, core_ids=list(range(NCORES)))
    out = np.empty((N, H), np.float32)
    for c in range(NCORES):
        out[c * NS:(c + 1) * NS] = res.results[c]["out_hT"].T
    return out
